# revision 42
# baseline (speedup 1.0000x reference)
# Multi-head attention (B=4, S=2048, D=512, H=8) on 8 Trainium2 NeuronCores.
#
# Sharding: core c handles batch c//2 and query rows [(c%2)*1024, (c%2+1)*1024)
# for all 8 heads over all 2048 keys. Output slices are disjoint -> no
# collectives needed.
#
# v2 design notes (vs the v1 baseline at 188us):
#   - mask machinery removed entirely: masked/padded keys ship ZERO xk/xv
#     columns and a 0 entry in the "kones" vector that fills the denominator
#     row of the v stationary. exp(q.0)=1 but both its v row and its ones
#     entry are 0, so padded keys contribute nothing to numerator or
#     denominator. Works for the compacted AND the dense fallback layout.
#   - k bias dropped (constant-per-query term, cancels exactly in softmax).
#     v bias + o bias folded on host: bo2 = bo + Wo @ bv (softmax rows sum
#     to 1, so the bv term rides through attention unchanged).
#   - v projection interleaved into the j=0 attention loop so the scalar
#     engine's exp stream (the true bottleneck, ~72 x 1us) starts ~15us
#     earlier and vproj hides under it.
#   - odd heads store v as [1 | v] so their attnv PSUM lands at partitions
#     63..127: the normalize multiply writes outTn[64:128] lane-aligned,
#     killing v1's 4 SBUF->SBUF shuffle DMAs.
#   - reciprocal: v1 spent 6.5us per [1,1024] DVE reciprocal (52us total!).
#     Now the two denominators of a head pair round-trip through DRAM into a
#     [128,16] tile and use one reciprocal_approx_fast (~0.2us).
#   - output projection runs in ONE wave: 8 x [128,512] single-bank PSUM
#     accumulators, jj-outer, so jj=0..2 passes overlap the last pair's
#     normalization; bias is added by the DVE during PSUM evacuation.
#   - PE HAM warmup: ~10 dummy matmuls during the initial DMA wait so the
#     clock gate is at 8/8 when real matmuls arrive; exp table preloaded the
#     same way.
#   - input DMAs spread across scalar/sync/vector/gpsimd rings, output DMAs
#     round-robin over 4 rings.

import sys
import os

for _p in ("/opt/trn_rl_repo", "/root/.axon_site/_ro/trn_rl_repo"):
    if os.path.isdir(_p) and _p not in sys.path:
        sys.path.append(_p)

import numpy as np

B, S, D, H = 4, 2048, 512, 8
DK = D // H          # 64
N_CORES = 8
SQ = S // 2          # 1024 query rows per core
SKC = 1152           # compacted key capacity (9 tiles of 128)

_compiled = {}       # skeys -> Bacc
last_results = None  # BassKernelResults of the most recent run (for test.py)


def _build(skeys):
    import concourse.bass as bass  # noqa: F401
    from concourse import bacc
    import concourse.tile as tile
    import concourse.mybir as mybir

    fp32 = mybir.dt.float32
    bf16 = mybir.dt.bfloat16
    EXP = mybir.ActivationFunctionType.Exp
    nkt = skeys // 128
    # key-side projection chunks of up to 512 columns
    kchunks = []
    off = 0
    while off < skeys:
        w = min(512, skeys - off)
        kchunks.append((off, w))
        off += w

    nc = bacc.Bacc("TRN2", target_bir_lowering=False, debug=False,
                   num_devices=N_CORES)

    xq = nc.dram_tensor("xq", [D, SQ], bf16, kind="ExternalInput")
    xk = nc.dram_tensor("xk", [D, skeys], bf16, kind="ExternalInput")
    xv = nc.dram_tensor("xv", [D, skeys], bf16, kind="ExternalInput")
    wq = nc.dram_tensor("wq", [D, D], bf16, kind="ExternalInput")
    wk = nc.dram_tensor("wk", [D, D], bf16, kind="ExternalInput")
    wv = nc.dram_tensor("wv", [D, D], bf16, kind="ExternalInput")
    wo = nc.dram_tensor("wo", [D, D], bf16, kind="ExternalInput")
    bq = nc.dram_tensor("bq", [128, 4], fp32, kind="ExternalInput")
    ko = nc.dram_tensor("ko", [128, nkt], bf16, kind="ExternalInput")
    bo2 = nc.dram_tensor("bo2", [1, D], fp32, kind="ExternalInput")
    bo2h = nc.dram_tensor("bo2h", [1, D], bf16, kind="ExternalInput")
    out = nc.dram_tensor("out", [D, SQ], bf16, kind="ExternalOutput")
    rds = nc.dram_tensor("rds", [H, SQ], fp32)   # scratch: denominators
    rds2 = nc.dram_tensor("rds2", [H, SQ], fp32)  # scratch: 1/denominator
    rds2b = nc.dram_tensor("rds2b", [H, SQ], bf16)  # 1/den in bf16

    with tile.TileContext(nc) as tc:
        with (
            tc.tile_pool(name="consts", bufs=1) as consts,
            tc.tile_pool(name="xin", bufs=1) as xin,
            tc.tile_pool(name="qk", bufs=1) as qk,
            tc.tile_pool(name="vp", bufs=1) as vp,
            tc.tile_pool(name="stp", bufs=6) as stp,
            tc.tile_pool(name="small", bufs=2) as small,
            tc.tile_pool(name="osb", bufs=8) as osb,
            tc.tile_pool(name="pst", bufs=2, space="PSUM") as pst,
            tc.tile_pool(name="pout", bufs=2, space="PSUM") as pout,
        ):
            # ---- input DMAs first (their triggers must not sit behind
            # the 2.7us exp-table load on the scalar queue) ----
            wq_sb = consts.tile([128, 4, D], bf16, tag="wq")
            nc.scalar.dma_start(out=wq_sb[:],
                                in_=wq.rearrange("(kc p) n -> p kc n", p=128))
            xq_sb = xin.tile([128, 4, SQ], bf16, tag="xq")
            for kc, ring in ((0, nc.sync), (1, nc.gpsimd), (2, nc.sync),
                             (3, nc.gpsimd)):
                ring.dma_start(
                    out=xq_sb[:, kc, :],
                    in_=xq[kc * 128:(kc + 1) * 128, :])
            wk_sb = consts.tile([128, 4, D], bf16, tag="wk")
            nc.scalar.dma_start(out=wk_sb[:],
                                in_=wk.rearrange("(kc p) n -> p kc n", p=128))
            xk_sb = xin.tile([128, 4, skeys], bf16, tag="xk")
            for kh, ring in ((0, nc.scalar), (1, nc.sync)):
                ring.dma_start(
                    out=xk_sb[:, 2 * kh:2 * kh + 2, :],
                    in_=xk[kh * 256:(kh + 1) * 256, :]
                    .rearrange("(kc p) s -> p kc s", p=128))
            bq_sb = consts.tile([128, 4], fp32, tag="bq")
            nc.gpsimd.dma_start(out=bq_sb[:], in_=bq[:, :])
            ko_sb = consts.tile([128, nkt], bf16, tag="ko")
            nc.gpsimd.dma_start(out=ko_sb[:], in_=ko[:, :])
            wv_sb = consts.tile([128, 4, D], bf16, tag="wv")
            nc.scalar.dma_start(out=wv_sb[:],
                                in_=wv.rearrange("(kc p) n -> p kc n", p=128))
            xv_sb = xin.tile([128, 4, skeys], bf16, tag="xv")
            for kh in range(2):
                nc.sync.dma_start(
                    out=xv_sb[:, 2 * kh:2 * kh + 2, :],
                    in_=xv[kh * 256:(kh + 1) * 256, :]
                    .rearrange("(kc p) s -> p kc s", p=128))
            # WoT rows packed by head pair: [128, 4, 512]
            wo_sb = consts.tile([128, 4, D], bf16, tag="wo")
            nc.scalar.dma_start(out=wo_sb[:],
                                in_=wo.rearrange("(j p) n -> p j n", p=128))
            bo2c_sb = consts.tile([128, 4], fp32, tag="bo2c")
            nc.gpsimd.dma_start(out=bo2c_sb[:],
                                in_=bo2[0:1, :].rearrange(
                                    "o (db p) -> (o p) db", p=128))
            onesb = consts.tile([128, 128], bf16, tag="onesb")
            nc.vector.memset(onesb[:], 1.0)

            # ---- warmup: exp table load + PE HAM un-throttle ----
            warm = consts.tile([128, 512], bf16, tag="warm")
            nc.vector.memset(warm[:], 0.25)
            warm_act = consts.tile([1, 512], bf16, tag="warma")
            nc.scalar.activation(out=warm_act[:], in_=warm[0:1, :], func=EXP,
                                 scale=1.0)
            # >=3.4us of sustained PE busy flips the HAM clock gate to
            # 8/8; bridge the input-DMA wait so the projections start warm
            wps = pst.tile([128, 1024], fp32, tag="st", name="warm_ps")
            for _ in range(12):
                nc.tensor.matmul(wps[:, 0:512], warm[:, 0:128],
                                 warm[:, 0:512], start=True, stop=True)

            # ---- q/k projections, interleaved per head pair ----
            # qproj psum on the "st" ring (scalar evacuates with the bias
            # add); kproj psum on the "po" ring (DVE evacuates) -> the two
            # rings double-buffer each other and the tensor engine never
            # waits for an evacuation.
            qT = qk.tile([128, 4, SQ], bf16, tag="qT")
            kT = qk.tile([128, 4, skeys], bf16, tag="kT")
            cgroups = [kchunks[i:i + 2] for i in range(0, len(kchunks), 2)]
            pq = [pst.tile([128, 1024], fp32, tag="st", name="pq0"),
                  pst.tile([128, 1024], fp32, tag="st", name="pq1"),
                  pout.tile([128, 1024], fp32, tag="po", name="pq2"),
                  pout.tile([128, 1024], fp32, tag="po", name="pq3")]
            for kc in range(4):
                for j in range(4):
                    for qc in range(2):
                        nc.tensor.matmul(
                            pq[j][:, qc * 512:(qc + 1) * 512],
                            wq_sb[:, kc, j * 128:(j + 1) * 128],
                            xq_sb[:, kc, qc * 512:(qc + 1) * 512],
                            start=(kc == 0), stop=(kc == 3))
            for j in (2, 3, 0, 1):
                for qc in range(2):
                    nc.scalar.add(qT[:, j, qc * 512:(qc + 1) * 512],
                                  pq[j][:, qc * 512:(qc + 1) * 512],
                                  bq_sb[:, j:j + 1])
            for j in range(4):
                for gi, grp in enumerate(cgroups):
                    pk = pout.tile([128, 1024], fp32, tag="po",
                                   name=f"pk_{j}_{gi}")
                    for kc in range(4):
                        for ci, (off, w) in enumerate(grp):
                            nc.tensor.matmul(
                                pk[:, ci * 512:ci * 512 + w],
                                wk_sb[:, kc, j * 128:(j + 1) * 128],
                                xk_sb[:, kc, off:off + w],
                                start=(kc == 0), stop=(kc == 3))
                    for ci, (off, w) in enumerate(grp):
                        nc.vector.tensor_copy(out=kT[:, j, off:off + w],
                                              in_=pk[:, ci * 512:ci * 512 + w])

            # ---- v stationaries: [v | kones] per head ----
            v_sb = vp.tile([128, nkt, H, DK + 1], bf16, tag="v")
            for h in range(H):
                nc.vector.tensor_copy(out=v_sb[:, :, h, DK], in_=ko_sb[:, :])
            for sk in range(nkt):
                pv = pst.tile([128, 1024], fp32, tag="st", name=f"pv_{sk}")
                for kc in range(4):
                    nc.tensor.matmul(
                        pv[:, 0:512],
                        xv_sb[:, kc, sk * 128:(sk + 1) * 128],
                        wv_sb[:, kc, :],
                        start=(kc == 0), stop=(kc == 3))
                nc.vector.tensor_copy(
                    out=v_sb[:, sk, :, 0:DK],
                    in_=pv[:, 0:512].rearrange("p (h m) -> p h m", h=H))

            # ---- attention, one head pair at a time ----
            # per sk: scores A -> attnvA(sk-1) -> scores B -> attnvB(sk-1);
            # exp of half A on the scalar engine (ACT table), half B on the
            # vector engine via a Schraudolph bit-trick:
            #   exp(s/8) ~ bf16_bitcast(int16(s*A + B)),
            #   A = 16*log2(e), B = 127*128 - 7
            # (max ~4% sawtooth error per element, pseudo-random across keys,
            # averages out in the attention sum; denominator uses the same
            # approximated values so the common mode cancels.)
            # Each pair's normalize chain is EMITTED inside the next pair's
            # sk loop so its DMA round trips never head-of-line-block the
            # vector engine's exp stream.
            SCH_A = 0.125 * 128.0 * 1.4426950408889634
            SCH_B = 127.0 * 128.0 - 7.0
            i16 = mybir.dt.int16
            outTn = qk.tile([128, 4, SQ], bf16, tag="outTn")
            norm_sched = {}

            for j in range(4):
                po0 = pout.tile([128, 1024], fp32, tag="po", name=f"po0_{j}")
                po1 = pout.tile([128, 1024], fp32, tag="po", name=f"po1_{j}")
                sts = []

                def attnv(sk, half, j=j, po0=po0, po1=po1, sts=sts):
                    mv = sts[sk][half]
                    po = po0 if half == 0 else po1
                    for qc in range(2):
                        nc.tensor.matmul(
                            po[0:DK + 1, qc * 512:(qc + 1) * 512],
                            v_sb[:, sk, 2 * j + half, :],
                            mv[:, qc * 512:(qc + 1) * 512],
                            start=(sk == 0), stop=(sk == nkt - 1))

                for sk in range(nkt):
                    psA = pst.tile([128, 1024], fp32, tag="st",
                                   name=f"psA_{j}_{sk}")
                    psB = pst.tile([128, 1024], fp32, tag="st",
                                   name=f"psB_{j}_{sk}")
                    for qc in range(2):
                        nc.tensor.matmul(
                            psA[:, qc * 512:(qc + 1) * 512],
                            kT[0:DK, j, sk * 128:(sk + 1) * 128],
                            qT[0:DK, j, qc * 512:(qc + 1) * 512],
                            start=True, stop=True, tile_position=(0, 0))
                    if sk > 0:
                        attnv(sk - 1, 0)
                    for qc in range(2):
                        nc.tensor.matmul(
                            psB[:, qc * 512:(qc + 1) * 512],
                            kT[DK:128, j, sk * 128:(sk + 1) * 128],
                            qT[DK:128, j, qc * 512:(qc + 1) * 512],
                            start=True, stop=True, tile_position=(64, 0))
                    if sk > 0:
                        attnv(sk - 1, 1)
                    stA = stp.tile([128, 1024], bf16, tag="stb",
                                   name=f"stA_{j}_{sk}")
                    nc.scalar.activation(out=stA[:], in_=psA[:], func=EXP,
                                         scale=0.125)
                    stB = stp.tile([128, 1024], i16, tag="stbi",
                                   name=f"stB_{j}_{sk}")
                    nc.vector.tensor_scalar(out=stB[:], in0=psB[:],
                                            scalar1=SCH_A, scalar2=SCH_B,
                                            op0=mybir.AluOpType.mult,
                                            op1=mybir.AluOpType.add)
                    sts.append((stA, stB[:, :].bitcast(bf16)))
                    # deferred pieces of the previous pair's normalize
                    for fn in norm_sched.pop((j, sk), []):
                        fn()
                if j == 3:
                    attnv(nkt - 1, 1)
                    attnv(nkt - 1, 0)
                else:
                    attnv(nkt - 1, 0)
                    attnv(nkt - 1, 1)

                # ---- evacuate; schedule normalize into the next pair ----
                if j == 3:
                    uB = small.tile([128, 1024], fp32, tag="u",
                                    name=f"uB_{j}")
                    nc.vector.tensor_copy(out=uB[0:DK + 1, :],
                                          in_=po1[0:DK + 1, :])
                    uA = small.tile([128, 1024], fp32, tag="u",
                                    name=f"uA_{j}")
                    nc.vector.tensor_copy(out=uA[0:DK + 1, :],
                                          in_=po0[0:DK + 1, :])
                else:
                    uA = small.tile([128, 1024], fp32, tag="u",
                                    name=f"uA_{j}")
                    nc.vector.tensor_copy(out=uA[0:DK + 1, :],
                                          in_=po0[0:DK + 1, :])
                    uB = small.tile([128, 1024], fp32, tag="u",
                                    name=f"uB_{j}")
                    nc.vector.tensor_copy(out=uB[0:DK + 1, :],
                                          in_=po1[0:DK + 1, :])

                if j < 3:
                    nc.sync.dma_start(out=rds[2 * j:2 * j + 1, :],
                                      in_=uA[DK:DK + 1, :])
                    nc.sync.dma_start(out=rds[2 * j + 1:2 * j + 2, :],
                                      in_=uB[DK:DK + 1, :])
                    den2 = small.tile([32, 2, 32], fp32, tag="den",
                                      name=f"den_{j}")
                    nc.gpsimd.dma_start(
                        out=den2[:],
                        in_=rds[2 * j:2 * j + 2, :]
                        .rearrange("h (p t) -> p h t", p=32))

                    def mk_recip(j=j, den2=den2):
                        rec2 = small.tile([32, 2, 32], fp32, tag="rec",
                                          name=f"rec_{j}")
                        nc.vector.reciprocal_approx_fast(out=rec2[:],
                                                         in_=den2[:])
                        nc.sync.dma_start(
                            out=rds2[2 * j:2 * j + 2, :]
                            .rearrange("h (p t) -> p h t", p=32),
                            in_=rec2[:])
                        bcA = small.tile([128, 1024], fp32, tag="bc",
                                         name=f"bcA_{j}")
                        nc.gpsimd.dma_start(
                            out=bcA[0:DK, :],
                            in_=rds2[2 * j:2 * j + 1, :]
                            .partition_broadcast(DK))
                        bcB = small.tile([128, 1024], fp32, tag="bc",
                                         name=f"bcB_{j}")
                        nc.gpsimd.dma_start(
                            out=bcB[0:DK, :],
                            in_=rds2[2 * j + 1:2 * j + 2, :]
                            .partition_broadcast(DK))
                        norm_sched.setdefault((j + 1, 4), []).append(
                            mk_mulA(j, bcA, 0))
                        norm_sched.setdefault((j + 1, 5), []).append(
                            mk_mulA(j, bcA, 1))
                        toddj = small.tile([DK, 1024], bf16, tag="todd",
                                           name=f"todd_{j}")
                        norm_sched.setdefault((j + 1, 6), []).append(
                            mk_mulB(j, bcB, 0, toddj))
                        norm_sched.setdefault((j + 1, 7), []).append(
                            mk_mulB(j, bcB, 1, toddj))

                    def mk_mulA(j, bcA, qc, uA=uA):
                        def mul():
                            sl = slice(qc * 512, (qc + 1) * 512)
                            nc.vector.tensor_mul(out=outTn[0:DK, j, sl],
                                                 in0=uA[0:DK, sl],
                                                 in1=bcA[0:DK, sl])
                        return mul

                    def mk_mulB(j, bcB, qc, todd, uB=uB):
                        def mul():
                            sl = slice(qc * 512, (qc + 1) * 512)
                            nc.vector.tensor_mul(out=todd[:, sl],
                                                 in0=uB[0:DK, sl],
                                                 in1=bcB[0:DK, sl])
                            ring = nc.sync if qc == 0 else nc.gpsimd
                            ring.dma_start(out=outTn[DK:128, j, sl],
                                           in_=todd[:, sl])
                        return mul
                    norm_sched.setdefault((j + 1, 2), []).append(mk_recip)

            # ---- pair 3 normalize: broadcast den through the PE (K=1 ones
            # matmul into the outproj PSUM banks), wide reciprocal, no DRAM
            # on the tail critical path ----
            P = [pst.tile([128, 1024], fp32, tag="st", name="pfA"),
                 pst.tile([128, 1024], fp32, tag="st", name="pfB"),
                 pout.tile([128, 1024], fp32, tag="po", name="pfC"),
                 pout.tile([128, 1024], fp32, tag="po", name="pfD")]
            for half, u, po in ((1, uB, po1), (0, uA, po0)):
                db = small.tile([128, 1024], bf16, tag="db",
                                name=f"db_{half}")
                nc.vector.tensor_copy(out=db[DK:DK + 1, :],
                                      in_=po[DK:DK + 1, :])
                bc_ps = P[2 + half]
                for qc in range(2):
                    nc.tensor.matmul(
                        bc_ps[0:DK, qc * 512:(qc + 1) * 512],
                        onesb[DK:DK + 1, 0:DK],
                        db[DK:DK + 1, qc * 512:(qc + 1) * 512],
                        start=True, stop=True)
                rq = small.tile([128, 1024], fp32, tag="rq",
                                name=f"rq_{half}")
                nc.scalar.copy(rq[0:DK, :], bc_ps[0:DK, :])
                rr = small.tile([128, 1024], fp32, tag="rr",
                                name=f"rr_{half}")
                nc.vector.reciprocal_approx_fast(out=rr[0:DK, :],
                                                 in_=rq[0:DK, :])
                if half == 0:
                    nc.vector.tensor_mul(out=outTn[0:DK, 3, :],
                                         in0=u[0:DK, :], in1=rr[0:DK, :])
                else:
                    todd = small.tile([DK, 1024], bf16, tag="todd",
                                      name="todd_3")
                    nc.vector.tensor_mul(out=todd[:], in0=u[0:DK, :],
                                         in1=rr[0:DK, :])
                    nc.sync.dma_start(out=outTn[DK:128, 3, 0:512],
                                      in_=todd[:, 0:512])
                    nc.gpsimd.dma_start(out=outTn[DK:128, 3, 512:1024],
                                        in_=todd[:, 512:1024])

            # ---- output projection, weight-stationary (output comes out
            # transposed [D, SQ]; the host transposes back for free). Each
            # LDW streams two N=512 matmuls and the bias is a per-partition
            # scalar add during evacuation -> no ones-matmuls.
            rings = [nc.sync, nc.gpsimd, nc.scalar]
            for jj in range(4):
                for db in range(4):
                    pf = P[db]
                    for qh in range(2):
                        nc.tensor.matmul(
                            pf[:, qh * 512:(qh + 1) * 512],
                            wo_sb[:, jj, db * 128:(db + 1) * 128],
                            outTn[:, jj, qh * 512:(qh + 1) * 512],
                            start=(jj == 0), stop=(jj == 3))
                    if jj == 3:
                        for qh in range(2):
                            k8 = db * 2 + qh
                            ob = osb.tile([128, 512], bf16, tag="ob",
                                          name=f"ob_{k8}")
                            if k8 % 2 == 0:
                                nc.vector.tensor_scalar_add(
                                    ob[:], pf[:, qh * 512:(qh + 1) * 512],
                                    bo2c_sb[:, db:db + 1])
                            else:
                                nc.scalar.add(
                                    ob[:], pf[:, qh * 512:(qh + 1) * 512],
                                    bo2c_sb[:, db:db + 1])
                            rings[k8 % 3].dma_start(
                                out=out[db * 128:(db + 1) * 128,
                                        qh * 512:(qh + 1) * 512],
                                in_=ob[:])

    nc.finalize()
    return nc


def _get_nc(skeys):
    if skeys not in _compiled:
        _compiled[skeys] = _build(skeys)
    return _compiled[skeys]


def kernel(query, key, value, key_padding_mask, Wq, bq, Wk, bk, Wv, bv,
           Wo, bo):
    global last_results
    from concourse.bass_utils import run_bass_kernel_spmd
    import ml_dtypes
    bf = ml_dtypes.bfloat16

    query = np.asarray(query, dtype=np.float32)
    key = np.asarray(key, dtype=np.float32)
    value = np.asarray(value, dtype=np.float32)
    mask = np.asarray(key_padding_mask).astype(bool)
    Wq = np.asarray(Wq, dtype=np.float32)
    Wk = np.asarray(Wk, dtype=np.float32)
    Wv = np.asarray(Wv, dtype=np.float32)
    Wo = np.asarray(Wo, dtype=np.float32)
    bqv = np.asarray(bq, dtype=np.float32)
    bvv = np.asarray(bv, dtype=np.float32)
    bov = np.asarray(bo, dtype=np.float32)

    # compact keys: keep only unmasked positions (zero-padded to SKC);
    # dense fallback when a batch keeps more than SKC. Masked/padded keys
    # carry zero v and a zero "ones" entry -> no mask bias needed anywhere.
    kept = [np.flatnonzero(~mask[b]) for b in range(B)]
    if max(len(k) for k in kept) <= SKC:
        skeys = SKC
        kc_l, vc_l, ko_l = [], [], []
        for b in range(B):
            n = len(kept[b])
            kc = np.zeros((skeys, D), np.float32)
            vc = np.zeros((skeys, D), np.float32)
            kc[:n] = key[b][kept[b]]
            vc[:n] = value[b][kept[b]]
            kones = np.zeros(skeys, np.float32)
            kones[:n] = 1.0
            kc_l.append(kc); vc_l.append(vc); ko_l.append(kones)
    else:
        skeys = S
        kc_l = [key[b] for b in range(B)]
        vc_l = [value[b] * (~mask[b])[:, None] for b in range(B)]
        ko_l = [(~mask[b]).astype(np.float32) for b in range(B)]

    nc = _get_nc(skeys)
    nkt = skeys // 128

    bo2 = bov + Wo @ bvv  # fold v bias through the output projection
    shared = {
        "wq": np.ascontiguousarray(Wq.T).astype(bf),
        "wk": np.ascontiguousarray(Wk.T).astype(bf),
        "wv": np.ascontiguousarray(Wv.T).astype(bf),
        "wo": np.ascontiguousarray(Wo.T).astype(bf),
        "bq": np.ascontiguousarray(bqv.reshape(4, 128).T),
        "bo2": bo2.reshape(1, D).astype(np.float32),
        "bo2h": bo2.reshape(1, D).astype(bf),
    }
    in_maps = []
    for c in range(N_CORES):
        b, qh = divmod(c, 2)
        qT = np.ascontiguousarray(query[b].T)
        m = {
            "xq": np.ascontiguousarray(
                qT[:, qh * SQ:(qh + 1) * SQ]).astype(bf),
            "xk": np.ascontiguousarray(kc_l[b].T).astype(bf),
            "xv": np.ascontiguousarray(vc_l[b].T).astype(bf),
            "ko": np.ascontiguousarray(
                ko_l[b].reshape(nkt, 128).T).astype(bf),
        }
        m.update(shared)
        in_maps.append(m)

    res = run_bass_kernel_spmd(nc, in_maps, list(range(N_CORES)))
    last_results = res

    out = np.empty((B, S, D), dtype=np.float32)
    for c in range(N_CORES):
        b, qh = divmod(c, 2)
        out[b, qh * SQ:(qh + 1) * SQ, :] = \
            res.results[c]["out"].T.astype(np.float32)
    return out


# revision 43
# speedup vs baseline: 1.0220x; 1.0220x over previous
# Multi-head attention (B=4, S=2048, D=512, H=8) on 8 Trainium2 NeuronCores.
#
# Sharding: core c handles batch c//2 and query rows [(c%2)*1024, (c%2+1)*1024)
# for all 8 heads over all 2048 keys. Output slices are disjoint -> no
# collectives needed.
#
# v2 design notes (vs the v1 baseline at 188us):
#   - mask machinery removed entirely: masked/padded keys ship ZERO xk/xv
#     columns and a 0 entry in the "kones" vector that fills the denominator
#     row of the v stationary. exp(q.0)=1 but both its v row and its ones
#     entry are 0, so padded keys contribute nothing to numerator or
#     denominator. Works for the compacted AND the dense fallback layout.
#   - k bias dropped (constant-per-query term, cancels exactly in softmax).
#     v bias + o bias folded on host: bo2 = bo + Wo @ bv (softmax rows sum
#     to 1, so the bv term rides through attention unchanged).
#   - v projection interleaved into the j=0 attention loop so the scalar
#     engine's exp stream (the true bottleneck, ~72 x 1us) starts ~15us
#     earlier and vproj hides under it.
#   - odd heads store v as [1 | v] so their attnv PSUM lands at partitions
#     63..127: the normalize multiply writes outTn[64:128] lane-aligned,
#     killing v1's 4 SBUF->SBUF shuffle DMAs.
#   - reciprocal: v1 spent 6.5us per [1,1024] DVE reciprocal (52us total!).
#     Now the two denominators of a head pair round-trip through DRAM into a
#     [128,16] tile and use one reciprocal_approx_fast (~0.2us).
#   - output projection runs in ONE wave: 8 x [128,512] single-bank PSUM
#     accumulators, jj-outer, so jj=0..2 passes overlap the last pair's
#     normalization; bias is added by the DVE during PSUM evacuation.
#   - PE HAM warmup: ~10 dummy matmuls during the initial DMA wait so the
#     clock gate is at 8/8 when real matmuls arrive; exp table preloaded the
#     same way.
#   - input DMAs spread across scalar/sync/vector/gpsimd rings, output DMAs
#     round-robin over 4 rings.

import sys
import os

for _p in ("/opt/trn_rl_repo", "/root/.axon_site/_ro/trn_rl_repo"):
    if os.path.isdir(_p) and _p not in sys.path:
        sys.path.append(_p)

import numpy as np

B, S, D, H = 4, 2048, 512, 8
DK = D // H          # 64
N_CORES = 8
SQ = S // 2          # 1024 query rows per core
SKC = 1152           # compacted key capacity (9 tiles of 128)

_compiled = {}       # skeys -> Bacc
last_results = None  # BassKernelResults of the most recent run (for test.py)


def _build(skeys):
    import concourse.bass as bass  # noqa: F401
    from concourse import bacc
    import concourse.tile as tile
    import concourse.mybir as mybir

    fp32 = mybir.dt.float32
    bf16 = mybir.dt.bfloat16
    EXP = mybir.ActivationFunctionType.Exp
    nkt = skeys // 128
    # key-side projection chunks of up to 512 columns
    kchunks = []
    off = 0
    while off < skeys:
        w = min(512, skeys - off)
        kchunks.append((off, w))
        off += w

    nc = bacc.Bacc("TRN2", target_bir_lowering=False, debug=False,
                   num_devices=N_CORES)

    xq = nc.dram_tensor("xq", [D, SQ], bf16, kind="ExternalInput")
    xk = nc.dram_tensor("xk", [D, skeys], bf16, kind="ExternalInput")
    xv = nc.dram_tensor("xv", [D, skeys], bf16, kind="ExternalInput")
    wq = nc.dram_tensor("wq", [D, D], bf16, kind="ExternalInput")
    wk = nc.dram_tensor("wk", [D, D], bf16, kind="ExternalInput")
    wv = nc.dram_tensor("wv", [D, D], bf16, kind="ExternalInput")
    wo = nc.dram_tensor("wo", [D, D], bf16, kind="ExternalInput")
    bq = nc.dram_tensor("bq", [128, 4], fp32, kind="ExternalInput")
    ko = nc.dram_tensor("ko", [128, nkt], bf16, kind="ExternalInput")
    bo2 = nc.dram_tensor("bo2", [1, D], fp32, kind="ExternalInput")
    bo2h = nc.dram_tensor("bo2h", [1, D], bf16, kind="ExternalInput")
    out = nc.dram_tensor("out", [D, SQ], bf16, kind="ExternalOutput")
    rds = nc.dram_tensor("rds", [H, SQ], fp32)   # scratch: denominators
    rds2 = nc.dram_tensor("rds2", [H, SQ], fp32)  # scratch: 1/denominator
    rds2b = nc.dram_tensor("rds2b", [H, SQ], bf16)  # 1/den in bf16

    with tile.TileContext(nc) as tc:
        with (
            tc.tile_pool(name="consts", bufs=1) as consts,
            tc.tile_pool(name="xin", bufs=1) as xin,
            tc.tile_pool(name="qk", bufs=1) as qk,
            tc.tile_pool(name="vp", bufs=1) as vp,
            tc.tile_pool(name="stp", bufs=6) as stp,
            tc.tile_pool(name="small", bufs=2) as small,
            tc.tile_pool(name="osb", bufs=8) as osb,
            tc.tile_pool(name="pst", bufs=2, space="PSUM") as pst,
            tc.tile_pool(name="pout", bufs=2, space="PSUM") as pout,
        ):
            # ---- input DMAs first (their triggers must not sit behind
            # the 2.7us exp-table load on the scalar queue) ----
            wq_sb = consts.tile([128, 4, D], bf16, tag="wq")
            nc.scalar.dma_start(out=wq_sb[:],
                                in_=wq.rearrange("(kc p) n -> p kc n", p=128))
            xq_sb = xin.tile([128, 4, SQ], bf16, tag="xq")
            for kc, ring in ((0, nc.sync), (1, nc.gpsimd), (2, nc.sync),
                             (3, nc.gpsimd)):
                ring.dma_start(
                    out=xq_sb[:, kc, :],
                    in_=xq[kc * 128:(kc + 1) * 128, :])
            wk_sb = consts.tile([128, 4, D], bf16, tag="wk")
            nc.scalar.dma_start(out=wk_sb[:],
                                in_=wk.rearrange("(kc p) n -> p kc n", p=128))
            xk_sb = xin.tile([128, 4, skeys], bf16, tag="xk")
            for kh, ring in ((0, nc.scalar), (1, nc.sync)):
                ring.dma_start(
                    out=xk_sb[:, 2 * kh:2 * kh + 2, :],
                    in_=xk[kh * 256:(kh + 1) * 256, :]
                    .rearrange("(kc p) s -> p kc s", p=128))
            bq_sb = consts.tile([128, 4], fp32, tag="bq")
            nc.gpsimd.dma_start(out=bq_sb[:], in_=bq[:, :])
            ko_sb = consts.tile([128, nkt], bf16, tag="ko")
            nc.gpsimd.dma_start(out=ko_sb[:], in_=ko[:, :])
            wv_sb = consts.tile([128, 4, D], bf16, tag="wv")
            nc.scalar.dma_start(out=wv_sb[:],
                                in_=wv.rearrange("(kc p) n -> p kc n", p=128))
            xv_sb = xin.tile([128, 4, skeys], bf16, tag="xv")
            for kh in range(2):
                nc.sync.dma_start(
                    out=xv_sb[:, 2 * kh:2 * kh + 2, :],
                    in_=xv[kh * 256:(kh + 1) * 256, :]
                    .rearrange("(kc p) s -> p kc s", p=128))
            # WoT rows packed by head pair: [128, 4, 512]
            wo_sb = consts.tile([128, 4, D], bf16, tag="wo")
            nc.scalar.dma_start(out=wo_sb[:],
                                in_=wo.rearrange("(j p) n -> p j n", p=128))
            bo2c_sb = consts.tile([128, 4], fp32, tag="bo2c")
            nc.gpsimd.dma_start(out=bo2c_sb[:],
                                in_=bo2[0:1, :].rearrange(
                                    "o (db p) -> (o p) db", p=128))
            onesb = consts.tile([128, 128], bf16, tag="onesb")
            nc.vector.memset(onesb[:], 1.0)

            # ---- warmup: exp table load + PE HAM un-throttle ----
            warm = consts.tile([128, 512], bf16, tag="warm")
            nc.vector.memset(warm[:], 0.25)
            warm_act = consts.tile([1, 512], bf16, tag="warma")
            nc.scalar.activation(out=warm_act[:], in_=warm[0:1, :], func=EXP,
                                 scale=1.0)
            wps = pst.tile([128, 1024], fp32, tag="st", name="warm_ps")
            for _ in range(6):
                nc.tensor.matmul(wps[:, 0:128], warm[:, 0:128],
                                 warm[:, 0:128], start=True, stop=True)

            # ---- q/k projections, interleaved per head pair ----
            # qproj psum on the "st" ring (scalar evacuates with the bias
            # add); kproj psum on the "po" ring (DVE evacuates) -> the two
            # rings double-buffer each other and the tensor engine never
            # waits for an evacuation.
            qT = qk.tile([128, 4, SQ], bf16, tag="qT")
            kT = qk.tile([128, 4, skeys], bf16, tag="kT")
            cgroups = [kchunks[i:i + 2] for i in range(0, len(kchunks), 2)]
            pq = [pst.tile([128, 1024], fp32, tag="st", name="pq0"),
                  pst.tile([128, 1024], fp32, tag="st", name="pq1"),
                  pout.tile([128, 1024], fp32, tag="po", name="pq2"),
                  pout.tile([128, 1024], fp32, tag="po", name="pq3")]
            for kc in range(4):
                for j in range(4):
                    for qc in range(2):
                        nc.tensor.matmul(
                            pq[j][:, qc * 512:(qc + 1) * 512],
                            wq_sb[:, kc, j * 128:(j + 1) * 128],
                            xq_sb[:, kc, qc * 512:(qc + 1) * 512],
                            start=(kc == 0), stop=(kc == 3))
            for j in (2, 3, 0, 1):
                for qc in range(2):
                    nc.scalar.add(qT[:, j, qc * 512:(qc + 1) * 512],
                                  pq[j][:, qc * 512:(qc + 1) * 512],
                                  bq_sb[:, j:j + 1])
            for j in range(4):
                for gi, grp in enumerate(cgroups):
                    pk = pout.tile([128, 1024], fp32, tag="po",
                                   name=f"pk_{j}_{gi}")
                    for kc in range(4):
                        for ci, (off, w) in enumerate(grp):
                            nc.tensor.matmul(
                                pk[:, ci * 512:ci * 512 + w],
                                wk_sb[:, kc, j * 128:(j + 1) * 128],
                                xk_sb[:, kc, off:off + w],
                                start=(kc == 0), stop=(kc == 3))
                    for ci, (off, w) in enumerate(grp):
                        nc.vector.tensor_copy(out=kT[:, j, off:off + w],
                                              in_=pk[:, ci * 512:ci * 512 + w])

            # ---- v stationaries: [v | kones] per head ----
            v_sb = vp.tile([128, nkt, H, DK + 1], bf16, tag="v")
            for h in range(H):
                nc.vector.tensor_copy(out=v_sb[:, :, h, DK], in_=ko_sb[:, :])
            for sk in range(nkt):
                pv = pst.tile([128, 1024], fp32, tag="st", name=f"pv_{sk}")
                for kc in range(4):
                    nc.tensor.matmul(
                        pv[:, 0:512],
                        xv_sb[:, kc, sk * 128:(sk + 1) * 128],
                        wv_sb[:, kc, :],
                        start=(kc == 0), stop=(kc == 3))
                nc.vector.tensor_copy(
                    out=v_sb[:, sk, :, 0:DK],
                    in_=pv[:, 0:512].rearrange("p (h m) -> p h m", h=H))

            # ---- attention, one head pair at a time ----
            # per sk: scores A -> attnvA(sk-1) -> scores B -> attnvB(sk-1);
            # exp of half A on the scalar engine (ACT table), half B on the
            # vector engine via a Schraudolph bit-trick:
            #   exp(s/8) ~ bf16_bitcast(int16(s*A + B)),
            #   A = 16*log2(e), B = 127*128 - 7
            # (max ~4% sawtooth error per element, pseudo-random across keys,
            # averages out in the attention sum; denominator uses the same
            # approximated values so the common mode cancels.)
            # Each pair's normalize chain is EMITTED inside the next pair's
            # sk loop so its DMA round trips never head-of-line-block the
            # vector engine's exp stream.
            SCH_A = 0.125 * 128.0 * 1.4426950408889634
            SCH_B = 127.0 * 128.0 - 7.0
            i16 = mybir.dt.int16
            outTn = qk.tile([128, 4, SQ], bf16, tag="outTn")
            norm_sched = {}

            for j in range(4):
                po0 = pout.tile([128, 1024], fp32, tag="po", name=f"po0_{j}")
                po1 = pout.tile([128, 1024], fp32, tag="po", name=f"po1_{j}")
                sts = []

                def attnv(sk, half, j=j, po0=po0, po1=po1, sts=sts):
                    mv = sts[sk][half]
                    po = po0 if half == 0 else po1
                    for qc in range(2):
                        nc.tensor.matmul(
                            po[0:DK + 1, qc * 512:(qc + 1) * 512],
                            v_sb[:, sk, 2 * j + half, :],
                            mv[:, qc * 512:(qc + 1) * 512],
                            start=(sk == 0), stop=(sk == nkt - 1))

                for sk in range(nkt):
                    psA = pst.tile([128, 1024], fp32, tag="st",
                                   name=f"psA_{j}_{sk}")
                    psB = pst.tile([128, 1024], fp32, tag="st",
                                   name=f"psB_{j}_{sk}")
                    for qc in range(2):
                        nc.tensor.matmul(
                            psA[:, qc * 512:(qc + 1) * 512],
                            kT[0:DK, j, sk * 128:(sk + 1) * 128],
                            qT[0:DK, j, qc * 512:(qc + 1) * 512],
                            start=True, stop=True, tile_position=(0, 0))
                    if sk > 0:
                        attnv(sk - 1, 0)
                    for qc in range(2):
                        nc.tensor.matmul(
                            psB[:, qc * 512:(qc + 1) * 512],
                            kT[DK:128, j, sk * 128:(sk + 1) * 128],
                            qT[DK:128, j, qc * 512:(qc + 1) * 512],
                            start=True, stop=True, tile_position=(64, 0))
                    if sk > 0:
                        attnv(sk - 1, 1)
                    stA = stp.tile([128, 1024], bf16, tag="stb",
                                   name=f"stA_{j}_{sk}")
                    nc.scalar.activation(out=stA[:], in_=psA[:], func=EXP,
                                         scale=0.125)
                    stB = stp.tile([128, 1024], i16, tag="stbi",
                                   name=f"stB_{j}_{sk}")
                    nc.vector.tensor_scalar(out=stB[:], in0=psB[:],
                                            scalar1=SCH_A, scalar2=SCH_B,
                                            op0=mybir.AluOpType.mult,
                                            op1=mybir.AluOpType.add)
                    sts.append((stA, stB[:, :].bitcast(bf16)))
                    # deferred pieces of the previous pair's normalize
                    for fn in norm_sched.pop((j, sk), []):
                        fn()
                if j == 3:
                    attnv(nkt - 1, 1)
                    attnv(nkt - 1, 0)
                else:
                    attnv(nkt - 1, 0)
                    attnv(nkt - 1, 1)

                # ---- evacuate; schedule normalize into the next pair ----
                if j == 3:
                    uB = small.tile([128, 1024], fp32, tag="u",
                                    name=f"uB_{j}")
                    nc.vector.tensor_copy(out=uB[0:DK + 1, :],
                                          in_=po1[0:DK + 1, :])
                    uA = small.tile([128, 1024], fp32, tag="u",
                                    name=f"uA_{j}")
                    nc.vector.tensor_copy(out=uA[0:DK + 1, :],
                                          in_=po0[0:DK + 1, :])
                else:
                    uA = small.tile([128, 1024], fp32, tag="u",
                                    name=f"uA_{j}")
                    nc.vector.tensor_copy(out=uA[0:DK + 1, :],
                                          in_=po0[0:DK + 1, :])
                    uB = small.tile([128, 1024], fp32, tag="u",
                                    name=f"uB_{j}")
                    nc.vector.tensor_copy(out=uB[0:DK + 1, :],
                                          in_=po1[0:DK + 1, :])

                if j < 3:
                    nc.sync.dma_start(out=rds[2 * j:2 * j + 1, :],
                                      in_=uA[DK:DK + 1, :])
                    nc.sync.dma_start(out=rds[2 * j + 1:2 * j + 2, :],
                                      in_=uB[DK:DK + 1, :])
                    den2 = small.tile([32, 2, 32], fp32, tag="den",
                                      name=f"den_{j}")
                    nc.gpsimd.dma_start(
                        out=den2[:],
                        in_=rds[2 * j:2 * j + 2, :]
                        .rearrange("h (p t) -> p h t", p=32))

                    def mk_recip(j=j, den2=den2):
                        rec2 = small.tile([32, 2, 32], fp32, tag="rec",
                                          name=f"rec_{j}")
                        nc.vector.reciprocal_approx_fast(out=rec2[:],
                                                         in_=den2[:])
                        nc.sync.dma_start(
                            out=rds2[2 * j:2 * j + 2, :]
                            .rearrange("h (p t) -> p h t", p=32),
                            in_=rec2[:])
                        bcA = small.tile([128, 1024], fp32, tag="bc",
                                         name=f"bcA_{j}")
                        nc.gpsimd.dma_start(
                            out=bcA[0:DK, :],
                            in_=rds2[2 * j:2 * j + 1, :]
                            .partition_broadcast(DK))
                        bcB = small.tile([128, 1024], fp32, tag="bc",
                                         name=f"bcB_{j}")
                        nc.gpsimd.dma_start(
                            out=bcB[0:DK, :],
                            in_=rds2[2 * j + 1:2 * j + 2, :]
                            .partition_broadcast(DK))
                        norm_sched.setdefault((j + 1, 4), []).append(
                            mk_mulA(j, bcA, 0))
                        norm_sched.setdefault((j + 1, 5), []).append(
                            mk_mulA(j, bcA, 1))
                        toddj = small.tile([DK, 1024], bf16, tag="todd",
                                           name=f"todd_{j}")
                        norm_sched.setdefault((j + 1, 6), []).append(
                            mk_mulB(j, bcB, 0, toddj))
                        norm_sched.setdefault((j + 1, 7), []).append(
                            mk_mulB(j, bcB, 1, toddj))

                    def mk_mulA(j, bcA, qc, uA=uA):
                        def mul():
                            sl = slice(qc * 512, (qc + 1) * 512)
                            nc.vector.tensor_mul(out=outTn[0:DK, j, sl],
                                                 in0=uA[0:DK, sl],
                                                 in1=bcA[0:DK, sl])
                        return mul

                    def mk_mulB(j, bcB, qc, todd, uB=uB):
                        def mul():
                            sl = slice(qc * 512, (qc + 1) * 512)
                            nc.vector.tensor_mul(out=todd[:, sl],
                                                 in0=uB[0:DK, sl],
                                                 in1=bcB[0:DK, sl])
                            ring = nc.sync if qc == 0 else nc.gpsimd
                            ring.dma_start(out=outTn[DK:128, j, sl],
                                           in_=todd[:, sl])
                        return mul
                    norm_sched.setdefault((j + 1, 2), []).append(mk_recip)

            # ---- pair 3 normalize: broadcast den through the PE (K=1 ones
            # matmul into the outproj PSUM banks), wide reciprocal, no DRAM
            # on the tail critical path ----
            P = [pst.tile([128, 1024], fp32, tag="st", name="pfA"),
                 pst.tile([128, 1024], fp32, tag="st", name="pfB"),
                 pout.tile([128, 1024], fp32, tag="po", name="pfC"),
                 pout.tile([128, 1024], fp32, tag="po", name="pfD")]
            for half, u, po in ((1, uB, po1), (0, uA, po0)):
                db = small.tile([128, 1024], bf16, tag="db",
                                name=f"db_{half}")
                nc.vector.tensor_copy(out=db[DK:DK + 1, :],
                                      in_=po[DK:DK + 1, :])
                bc_ps = P[2 + half]
                for qc in range(2):
                    nc.tensor.matmul(
                        bc_ps[0:DK, qc * 512:(qc + 1) * 512],
                        onesb[DK:DK + 1, 0:DK],
                        db[DK:DK + 1, qc * 512:(qc + 1) * 512],
                        start=True, stop=True)
                rq = small.tile([128, 1024], fp32, tag="rq",
                                name=f"rq_{half}")
                nc.scalar.copy(rq[0:DK, :], bc_ps[0:DK, :])
                rr = small.tile([128, 1024], fp32, tag="rr",
                                name=f"rr_{half}")
                nc.vector.reciprocal_approx_fast(out=rr[0:DK, :],
                                                 in_=rq[0:DK, :])
                if half == 0:
                    nc.vector.tensor_mul(out=outTn[0:DK, 3, :],
                                         in0=u[0:DK, :], in1=rr[0:DK, :])
                else:
                    todd = small.tile([DK, 1024], bf16, tag="todd",
                                      name="todd_3")
                    nc.vector.tensor_mul(out=todd[:], in0=u[0:DK, :],
                                         in1=rr[0:DK, :])
                    nc.sync.dma_start(out=outTn[DK:128, 3, 0:512],
                                      in_=todd[:, 0:512])
                    nc.gpsimd.dma_start(out=outTn[DK:128, 3, 512:1024],
                                        in_=todd[:, 512:1024])

            # ---- output projection, weight-stationary (output comes out
            # transposed [D, SQ]; the host transposes back for free). Each
            # LDW streams two N=512 matmuls and the bias is a per-partition
            # scalar add during evacuation -> no ones-matmuls.
            rings = [nc.sync, nc.gpsimd, nc.scalar]
            for jj in range(4):
                for db in range(4):
                    pf = P[db]
                    for qh in range(2):
                        nc.tensor.matmul(
                            pf[:, qh * 512:(qh + 1) * 512],
                            wo_sb[:, jj, db * 128:(db + 1) * 128],
                            outTn[:, jj, qh * 512:(qh + 1) * 512],
                            start=(jj == 0), stop=(jj == 3))
                    if jj == 3:
                        for qh in range(2):
                            k8 = db * 2 + qh
                            ob = osb.tile([128, 512], bf16, tag="ob",
                                          name=f"ob_{k8}")
                            if k8 % 2 == 0:
                                nc.vector.tensor_scalar_add(
                                    ob[:], pf[:, qh * 512:(qh + 1) * 512],
                                    bo2c_sb[:, db:db + 1])
                            else:
                                nc.scalar.add(
                                    ob[:], pf[:, qh * 512:(qh + 1) * 512],
                                    bo2c_sb[:, db:db + 1])
                            rings[k8 % 3].dma_start(
                                out=out[db * 128:(db + 1) * 128,
                                        qh * 512:(qh + 1) * 512],
                                in_=ob[:])

    nc.finalize()
    return nc


def _get_nc(skeys):
    if skeys not in _compiled:
        _compiled[skeys] = _build(skeys)
    return _compiled[skeys]


def kernel(query, key, value, key_padding_mask, Wq, bq, Wk, bk, Wv, bv,
           Wo, bo):
    global last_results
    from concourse.bass_utils import run_bass_kernel_spmd
    import ml_dtypes
    bf = ml_dtypes.bfloat16

    query = np.asarray(query, dtype=np.float32)
    key = np.asarray(key, dtype=np.float32)
    value = np.asarray(value, dtype=np.float32)
    mask = np.asarray(key_padding_mask).astype(bool)
    Wq = np.asarray(Wq, dtype=np.float32)
    Wk = np.asarray(Wk, dtype=np.float32)
    Wv = np.asarray(Wv, dtype=np.float32)
    Wo = np.asarray(Wo, dtype=np.float32)
    bqv = np.asarray(bq, dtype=np.float32)
    bvv = np.asarray(bv, dtype=np.float32)
    bov = np.asarray(bo, dtype=np.float32)

    # compact keys: keep only unmasked positions (zero-padded to SKC);
    # dense fallback when a batch keeps more than SKC. Masked/padded keys
    # carry zero v and a zero "ones" entry -> no mask bias needed anywhere.
    kept = [np.flatnonzero(~mask[b]) for b in range(B)]
    if max(len(k) for k in kept) <= SKC:
        skeys = SKC
        kc_l, vc_l, ko_l = [], [], []
        for b in range(B):
            n = len(kept[b])
            kc = np.zeros((skeys, D), np.float32)
            vc = np.zeros((skeys, D), np.float32)
            kc[:n] = key[b][kept[b]]
            vc[:n] = value[b][kept[b]]
            kones = np.zeros(skeys, np.float32)
            kones[:n] = 1.0
            kc_l.append(kc); vc_l.append(vc); ko_l.append(kones)
    else:
        skeys = S
        kc_l = [key[b] for b in range(B)]
        vc_l = [value[b] * (~mask[b])[:, None] for b in range(B)]
        ko_l = [(~mask[b]).astype(np.float32) for b in range(B)]

    nc = _get_nc(skeys)
    nkt = skeys // 128

    bo2 = bov + Wo @ bvv  # fold v bias through the output projection
    shared = {
        "wq": np.ascontiguousarray(Wq.T).astype(bf),
        "wk": np.ascontiguousarray(Wk.T).astype(bf),
        "wv": np.ascontiguousarray(Wv.T).astype(bf),
        "wo": np.ascontiguousarray(Wo.T).astype(bf),
        "bq": np.ascontiguousarray(bqv.reshape(4, 128).T),
        "bo2": bo2.reshape(1, D).astype(np.float32),
        "bo2h": bo2.reshape(1, D).astype(bf),
    }
    in_maps = []
    for c in range(N_CORES):
        b, qh = divmod(c, 2)
        qT = np.ascontiguousarray(query[b].T)
        m = {
            "xq": np.ascontiguousarray(
                qT[:, qh * SQ:(qh + 1) * SQ]).astype(bf),
            "xk": np.ascontiguousarray(kc_l[b].T).astype(bf),
            "xv": np.ascontiguousarray(vc_l[b].T).astype(bf),
            "ko": np.ascontiguousarray(
                ko_l[b].reshape(nkt, 128).T).astype(bf),
        }
        m.update(shared)
        in_maps.append(m)

    res = run_bass_kernel_spmd(nc, in_maps, list(range(N_CORES)))
    last_results = res

    out = np.empty((B, S, D), dtype=np.float32)
    for c in range(N_CORES):
        b, qh = divmod(c, 2)
        out[b, qh * SQ:(qh + 1) * SQ, :] = \
            res.results[c]["out"].T.astype(np.float32)
    return out
